# revision 1
# baseline (speedup 1.0000x reference)
"""LocalRNN Trainium2 kernel: GLU -> pointwise conv -> 9-step windowed LSTM.

Full inputs in, full output out. Sharding: batch across 8 cores (2 batches/core).

Design notes (evolved from the 375us fp32r baseline; now ~322us):
- All matmul operands bf16 (same 1 col/cycle PE stream rate as fp32r, but
  FWL weight loads, half the DMA bytes, 2x DVE elementwise, half SBUF).
  PSUM accumulation stays fp32; measured end-to-end rel err ~1.1e-2 (<2e-2).
- x is transposed and cast to bf16 on the host -> no PE transposes in prep;
  GLU runs directly on [channel, token] tiles.
- Conv (kernel_size=1) folded into W_ih on the host: G = (W_ih@conv_w) @ u
  with bias b_ih+b_hh+W_ih@conv_b.  Gate rows permuted host-side to I,F,O,G.
- gt (input-side gates, one per token, 9x reuse across windows) lives in one
  big [128, 16*1040] bf16 tile; pad columns land by DMA; per-step slices
  enter PSUM via 16 CONCURRENT 32x32 tile_position matmuls (gt row blocks
  stored rotated r=(c+q)%4 so all 16 (row_grp, col_grp) tiles are distinct),
  ~2x faster than 4 full-array identity matmuls per unit.
- Step 0 (h=0) runs as inject-only units + cells straight from PSUM.
- PE warm-up burst at t=0 (HAM clock gate needs ~3.4us of busy to unthrottle)
  plus filler matmuls across the prep->steady-state transition.
- Input DMAs on the sync queue (scalar-queue issues would stall ACT);
  output DMA'd in bf16 transposed layout split across both hwdge queues
  (one per 128-row block, 2KB/partition packets); host transposes back.
"""
from contextlib import ExitStack

import numpy as np
import ml_dtypes

import concourse.bass as bass
import concourse.mybir as mybir
import concourse.tile as tile
from concourse import bacc, bass_utils
from concourse.masks import make_identity

F32 = mybir.dt.float32
BF16 = mybir.dt.bfloat16
AF = mybir.ActivationFunctionType

N_CORES = 8
B_PER_CORE = 2          # batches per core
L = 512                 # sequence length
NT = B_PER_CORE * L     # tokens per core = 1024
D = 512                 # model dim
DH = 256                # GLU half dim
G4 = 4 * D              # 2048 gate rows
K = 9                   # window size
PAD = K - 1             # 8
LW = PAD + L            # 520: per-(tile,batch) padded G row width
TW = B_PER_CORE * LW    # 1040: per-tile width in the big gt tile

_cache = {}


def _build():
    nc = bacc.Bacc(
        trn_type="TRN2", target_bir_lowering=False, debug=False, num_devices=N_CORES
    )

    xt_d = nc.dram_tensor("xt", [D, NT], BF16, kind="ExternalInput").ap()     # x transposed, a rows 0:256, b rows 256:512
    wf_d = nc.dram_tensor("wf", [DH, G4], BF16, kind="ExternalInput").ap()    # (w_ih@conv_w).T permuted
    whh_d = nc.dram_tensor("whh", [D, G4], BF16, kind="ExternalInput").ap()   # w_hh.T permuted
    bias_d = nc.dram_tensor("bias", [128, 32], F32, kind="ExternalInput").ap()
    pad_d = nc.dram_tensor("gtpad", [128, 16 * 2 * PAD], BF16, kind="ExternalInput").ap()
    out_d = nc.dram_tensor("out", [D, NT], BF16, kind="ExternalOutput").ap()  # transposed out

    with tile.TileContext(nc) as tc, ExitStack() as top:
        const_pool = top.enter_context(tc.tile_pool(name="const", bufs=1))
        w_pool = top.enter_context(tc.tile_pool(name="weights", bufs=1))
        state_pool = top.enter_context(tc.tile_pool(name="state", bufs=1))

        z512 = const_pool.tile([128, 512], BF16, tag="z512")
        nc.gpsimd.memset(z512[:], 0.0)
        ident = const_pool.tile([128, 128], BF16, tag="id")
        make_identity(nc, ident[:])
        bias_sb = const_pool.tile([128, 32], F32, tag="bias")
        nc.sync.dma_start(bias_sb[:], bias_d)

        # input DMAs split across both hwdge queues (sync + scalar), ordered
        # so GLU inputs land first, then wf, then whh
        xt = [w_pool.tile([128, NT], BF16, tag=f"xt{t}", name=f"xt{t}")
              for t in range(4)]
        wf = [w_pool.tile([128, G4], BF16, tag=f"wf{ck}", name=f"wf{ck}")
              for ck in range(2)]
        whh = [w_pool.tile([128, G4], BF16, tag=f"whh{dk}", name=f"whh{dk}")
               for dk in range(4)]
        # all input DMAs on the sync queue: the scalar queue shares the ACT
        # engine FIFO and issue slots there would delay the GLU sigmoids
        for t in (2, 3, 0, 1):
            nc.sync.dma_start(xt[t][:], xt_d[t * 128:(t + 1) * 128, :])
        for ck in range(2):
            nc.sync.dma_start(wf[ck][:], wf_d[ck * 128:(ck + 1) * 128, :])
        for dk in range(4):
            nc.sync.dma_start(whh[dk][:], whh_d[dk * 128:(dk + 1) * 128, :])

        # big gt table: 16 gate-row tiles x (2 batches x 520)
        gt = state_pool.tile([128, 16 * TW], BF16, tag="gt", name="gt")
        hT = [[state_pool.tile([128, NT], BF16, tag=f"h{p}_{j}", name=f"h{p}_{j}")
               for j in range(4)] for p in range(2)]
        cT = [state_pool.tile([128, NT], BF16, tag=f"c{j}", name=f"c{j}") for j in range(4)]
        uT = [state_pool.tile([128, NT], BF16, tag=f"uT{ci}", name=f"uT{ci}")
              for ci in range(2)]

        tp = top.enter_context(tc.tile_pool(name="tmp", bufs=2))
        # one uniform PSUM pool for the whole kernel: 2 slots x 4 banks
        psg = top.enter_context(tc.tile_pool(name="psg", bufs=2, space="PSUM"))

        def warm(n):
            """n dummy N=512 matmuls to trip/hold the HAM clock gate."""
            for _ in range(n):
                P = psg.tile([128, G4], F32, tag="P", name="Pw")
                for q in range(4):
                    nc.tensor.matmul(
                        P[:, q * 512:(q + 1) * 512], ident[:], z512[:],
                        start=True, stop=True,
                    )

        # pad columns (bias-only virtual tokens) land by DMA straight into
        # the strided pad slots of the big gt tile
        pad_view = gt[:].rearrange("p (i b c) -> p i b c", i=16, b=2)[:, :, :, 0:PAD]
        nc.sync.dma_start(pad_view, pad_d)

        warm(3)  # ~12 MMs on cold clock trip the HAM gate, overlap input DMAs

        # GLU on transposed x: uT[ci] = xt[ci] * sigmoid(xt[2+ci])
        for ci in range(2):
            sgt = tp.tile([128, NT], BF16, tag="tSig", name="sgt")
            nc.scalar.activation(sgt[:], xt[2 + ci][:], AF.Sigmoid)
            nc.vector.tensor_mul(uT[ci][:], xt[ci][:], sgt[:])

        def g_phase(b, act_share):
            """act_share: how many of the 4 per-j gt writes go to ACT (the
            rest to DVE) — balances P-slot consumption against PE production."""
            for j in range(4):
                P = psg.tile([128, G4], F32, tag="P", name="Pg")
                for q in range(4):
                    for ck in range(2):
                        nc.tensor.matmul(
                            P[:, q * 512:(q + 1) * 512],
                            wf[ck][:, (4 * q + j) * 128:(4 * q + j + 1) * 128],
                            uT[ck][:, b * 512:(b + 1) * 512],
                            start=(ck == 0), stop=(ck == 1),
                        )
                for q in range(4):
                    i = 4 * q + j
                    dst = gt[:, i * TW + b * LW + PAD:i * TW + b * LW + LW]
                    if q < act_share:
                        nc.scalar.activation(
                            dst, P[:, q * 512:(q + 1) * 512],
                            AF.Identity, bias=bias_sb[:, i:i + 1],
                        )
                    else:
                        nc.vector.tensor_scalar_add(
                            dst, P[:, q * 512:(q + 1) * 512],
                            bias_sb[:, i:i + 1],
                        )

        def inject(P, b, j, k, stop):
            """gt -> PSUM via 16 concurrent 32x32 diagonal tile matmuls.

            gt tile i=4q+j stores logical row block c at partition block
            r=(c+q)%4 (host rotates wf/bias to match), so all 16 (q,c)
            tiles hit distinct (row_grp, col_grp) array tiles and stream
            concurrently (~one MM duration for the whole injection).
            """
            for q in range(4):
                off = (4 * q + j) * TW + b * LW + k
                for c in range(4):
                    r = (c + q) % 4
                    nc.tensor.matmul(
                        P[32 * c:32 * c + 32, q * 512:(q + 1) * 512],
                        ident[32 * r:32 * r + 32, 32 * r:32 * r + 32],
                        gt[32 * r:32 * r + 32, off:off + 512],
                        start=True, stop=stop,
                        tile_position=(32 * r, 32 * c),
                    )

        def unit0(b, j):
            """step 0 (h=0): inject gates, then c = sig(I)*tanh(G),
            h = sig(O)*tanh(c) straight from PSUM."""
            P = psg.tile([128, G4], F32, tag="P", name="P0")
            inject(P, b, j, 0, stop=True)
            cs = cT[j][:, b * 512:(b + 1) * 512]
            hs = hT[0][j][:, b * 512:(b + 1) * 512]
            # one strided sigmoid covers I [0:512] and O [1024:1536]
            tIO = tp.tile([128, 1024], BF16, tag="tSig", name="tIO0")
            pio = P[:, 0:1536].rearrange("p (r c) -> p r c", r=3)[:, 0:3:2, :]
            nc.scalar.activation(tIO[:], pio, AF.Sigmoid)
            tG = tp.tile([128, 512], BF16, tag="tG", name="tG0")
            nc.scalar.activation(tG[:], P[:, 1536:2048], AF.Tanh)
            nc.vector.tensor_mul(cs, tIO[:, 0:512], tG[:])
            tTc = tp.tile([128, 512], BF16, tag="tTc", name="tTc0")
            nc.scalar.activation(tTc[:], cs, AF.Tanh)
            nc.vector.tensor_mul(hs, tIO[:, 512:1024], tTc[:])

        def cell(j, b, P, k):
            """steps 1..8: full LSTM cell from psum P [128, 2048] = I|F|O|G."""
            cs = cT[j][:, b * 512:(b + 1) * 512]
            hs = hT[k % 2][j][:, b * 512:(b + 1) * 512]
            tSig = tp.tile([128, 1536], BF16, tag="tSig", name="tSig")
            nc.scalar.activation(tSig[:], P[:, 0:1536], AF.Sigmoid)
            tG = tp.tile([128, 512], BF16, tag="tG", name="tG")
            nc.scalar.activation(tG[:], P[:, 1536:2048], AF.Tanh)
            t1 = tp.tile([128, 512], BF16, tag="t1", name="t1")
            nc.vector.tensor_mul(t1[:], tSig[:, 0:512], tG[:])
            t2 = tp.tile([128, 512], BF16, tag="tG", name="t2")
            nc.vector.tensor_mul(t2[:], tSig[:, 512:1024], cs)
            nc.vector.tensor_add(cs, t1[:], t2[:])
            tTc = tp.tile([128, 512], BF16, tag="tTc", name="tTc")
            nc.scalar.activation(tTc[:], cs, AF.Tanh)
            nc.vector.tensor_mul(hs, tSig[:, 1024:1536], tTc[:])
            if k == K - 1 and b == B_PER_CORE - 1:
                # both batches of row-block j done: one DMA, 2KB/partition
                eng = nc.sync if j % 2 == 0 else nc.scalar
                eng.dma_start(out_d[j * 128:(j + 1) * 128, :], hT[k % 2][j][:])

        def unit(k, b, j):
            P = psg.tile([128, G4], F32, tag="P", name="P")
            # G slice first (ready early; starts each bank's group)
            inject(P, b, j, k, stop=False)
            for q in range(4):
                for dk in range(4):
                    nc.tensor.matmul(
                        P[:, q * 512:(q + 1) * 512],
                        whh[dk][:, (4 * q + j) * 128:(4 * q + j + 1) * 128],
                        hT[(k + 1) % 2][dk][:, b * 512:(b + 1) * 512],
                        start=False, stop=(dk == 3),
                    )
            cell(j, b, P[:], k)

        # prep order keeps PE fed: g_phase(0) -> unit0(b0) while ACT/DVE
        # run the step-0 cells, PE continues with g_phase(1) + filler.
        g_phase(0, act_share=3)   # ACT is idle here: split P drain DVE/ACT
        for j in range(4):
            unit0(0, j)
        g_phase(1, act_share=1)   # ACT mostly busy with step-0 cells
        for j in range(4):
            unit0(1, j)
        warm(1)  # filler across the step-0 cell dependency gap

        # ---------------- LSTM steps 1..8 ----------------
        for k in range(1, K):
            for b in range(B_PER_CORE):
                for j in range(4):
                    unit(k, b, j)

    nc.compile()
    return nc


def _make_in_maps(inputs):
    x = np.asarray(inputs["x"], dtype=np.float32)
    conv_w = np.asarray(inputs["conv_w"], dtype=np.float64)
    conv_b = np.asarray(inputs["conv_b"], dtype=np.float64)
    w_ih = np.asarray(inputs["w_ih"], dtype=np.float64)
    w_hh = np.asarray(inputs["w_hh"], dtype=np.float32)
    b_ih = np.asarray(inputs["b_ih"], dtype=np.float64)
    b_hh = np.asarray(inputs["b_hh"], dtype=np.float64)

    # gate permutation: torch order i,f,g,o -> i,f,o,g
    perm = np.concatenate([
        np.arange(0, D), np.arange(D, 2 * D),
        np.arange(3 * D, 4 * D), np.arange(2 * D, 3 * D),
    ])
    wf = (w_ih @ conv_w)[perm]                                  # [2048, 256]
    bias_mm = (b_ih + b_hh + w_ih @ conv_b)[perm]               # real columns
    bias_pad = (b_ih + b_hh)[perm]                              # zero-padded columns
    whh_p = w_hh[perm]

    # rotate 32-row blocks within each gate tile i (q = i//4): logical block
    # c is stored at partition block (c+q)%4, enabling the 16-way diagonal
    # tile_position injection on the device
    rot = np.empty(G4, dtype=np.int64)
    for i in range(16):
        q = i // 4
        for c in range(4):
            r = (c + q) % 4
            rot[i * 128 + 32 * r: i * 128 + 32 * r + 32] = \
                np.arange(i * 128 + 32 * c, i * 128 + 32 * c + 32)
    wf = wf[rot]
    bias_mm = bias_mm[rot]
    bias_pad = bias_pad[rot]

    bias_both = np.concatenate([
        bias_mm.astype(np.float32).reshape(16, 128).T,
        bias_pad.astype(np.float32).reshape(16, 128).T,
    ], axis=1)                                                  # [128, 32]
    # pad-column fill: stored (rotated) bias_pad broadcast over (b, c)
    bp128 = bias_pad.astype(ml_dtypes.bfloat16).reshape(16, 128).T      # [128, 16]
    gtpad = np.ascontiguousarray(
        np.broadcast_to(bp128[:, :, None], (128, 16, 2 * PAD)).reshape(128, -1)
    )
    shared = {
        "wf": np.ascontiguousarray(wf.T.astype(ml_dtypes.bfloat16)),     # [256, 2048]
        "whh": np.ascontiguousarray(whh_p.T.astype(ml_dtypes.bfloat16)), # [512, 2048]
        "bias": np.ascontiguousarray(bias_both),
        "gtpad": gtpad,
    }
    in_maps = []
    for c in range(N_CORES):
        m = dict(shared)
        xc = x[c * B_PER_CORE:(c + 1) * B_PER_CORE].reshape(NT, D)
        m["xt"] = np.ascontiguousarray(xc.T.astype(ml_dtypes.bfloat16))  # [512, 1024]
        in_maps.append(m)
    return in_maps


def kernel(x, conv_w, conv_b, w_ih, w_hh, b_ih, b_hh):
    if "nc" not in _cache:
        _cache["nc"] = _build()
    nc = _cache["nc"]

    in_maps = _make_in_maps(dict(
        x=x, conv_w=conv_w, conv_b=conv_b, w_ih=w_ih, w_hh=w_hh,
        b_ih=b_ih, b_hh=b_hh,
    ))

    res = bass_utils.run_bass_kernel_spmd(nc, in_maps, core_ids=list(range(N_CORES)))
    out = np.concatenate(
        [np.ascontiguousarray(np.asarray(r["out"]).astype(np.float32).T)
         .reshape(B_PER_CORE, L, D)
         for r in res.results], axis=0
    )
    return out

